# revision 39
# baseline (speedup 1.0000x reference)
"""Trainium2 Bass kernel for ChannelPredictor (dense_mlp).

Math (per batch sample, N=t*h*w=1024 tokens, D=NV=512, NC=4 channels):
  x = LayerNorm_d(yl)                       # (N, D), token-major in ref
  for k in 0..3:
    u_k = x @ Uk[:, :D].T + sum_{j<k} Uk[:, D+j*NV + idx_j].T + Uk_b
    o_k = relu(u_k) @ P.T + P_b

Reformulations used here:
  * one-hot @ Uk_w == gather of Uk_w columns -> dma_gather of bf16 embedding
    rows (tables pre-transposed host-side to (vocab, D) row-major), written
    d-major across partitions (transpose=True) so they add directly onto the
    matmul PSUM tiles.
  * the x-parts of all four U_k share rhs x -> one fused (D -> 4D) matmul.
  * LayerNorm stats via ones-vector matmuls (bf16 data, f32 accumulate);
    per-token mu/rstd broadcast across partitions via K=1 fp32 matmuls.

Sharding: data-parallel over batch b: 16 samples / 8 cores = 2 samples
(2048 tokens) per core; every core holds the full weights.

Schedule: all gathers are emitted first (GPSIMD/SWDGE runs them during the
LayerNorm lead-in); LayerNorm and the U/P matmul stages are interleaved per
512-token chunk so the PE pipeline fills early and stays warm.
"""

import sys

for _p in ("/opt/trn_rl_repo",):
    if _p not in sys.path:
        sys.path.insert(0, _p)

import numpy as np
import ml_dtypes
from contextlib import ExitStack

import concourse.bass as bass
import concourse.bacc as bacc
import concourse.mybir as mybir
import concourse.tile as tile
from concourse import bass_utils

F32 = mybir.dt.float32
BF16 = mybir.dt.bfloat16
I16 = mybir.dt.int16
AF = mybir.ActivationFunctionType
ALU = mybir.AluOpType
BF16NP = ml_dtypes.bfloat16

NCORES = 8
B, D, NV, NCH = 16, 512, 512, 4
T, H, W = 4, 16, 16
NTOK = T * H * W          # tokens per sample
BLOC = B // NCORES        # samples per core
TOK = BLOC * NTOK         # tokens per core
CH = 1024                 # token chunk (double-width PSUM tiles)
MH = 512                  # matmul moving-dim half of a chunk
NCHUNK = TOK // CH
EPS = 1e-5
PAIRS = [(1, 0), (2, 0), (2, 1), (3, 0), (3, 1), (3, 2)]  # (k, j) with j < k


def build_program():
    nc = bacc.Bacc("TRN2", target_bir_lowering=False, debug=False)

    x_d = nc.dram_tensor("x", (4, NCHUNK, 128, CH), F32, kind="ExternalInput")
    wu_d = nc.dram_tensor("wu", (4, 128, 4 * D), BF16, kind="ExternalInput")
    wp_d = nc.dram_tensor("wp", (4, 128, NV), BF16, kind="ExternalInput")
    tab_d = {
        (k, j): nc.dram_tensor(f"tab{k}{j}", (NV, D), BF16, kind="ExternalInput")
        for (k, j) in PAIRS
    }
    idx_d = nc.dram_tensor("idx", (3, 128, TOK // 16), I16, kind="ExternalInput")
    ub_d = nc.dram_tensor("ub", (128, 16), F32, kind="ExternalInput")
    pb_d = nc.dram_tensor("pb", (128, 4), F32, kind="ExternalInput")
    gb_d = nc.dram_tensor("gb", (128, 8), F32, kind="ExternalInput")
    out_d = [
        nc.dram_tensor(f"out{k}", (4, NCHUNK, 128, CH), F32, kind="ExternalOutput")
        for k in range(NCH)
    ]

    with tile.TileContext(nc) as tc, ExitStack() as ctx:
        const = ctx.enter_context(tc.tile_pool(name="const", bufs=1))

        idx_sb = []
        for j in range(3):
            it = const.tile([128, TOK // 16], I16, name=f"idx{j}")
            nc.gpsimd.dma_start(it[:], idx_d.ap()[j])
            idx_sb.append(it)

        # x loads first: the LayerNorm lead-in is the critical path.
        xpool = ctx.enter_context(tc.tile_pool(name="xpool", bufs=2))
        xcs = []
        for c in range(NCHUNK):
            xc = []
            for kt in range(4):
                xt = xpool.tile([128, CH], F32, tag=f"x{kt}", name=f"x_{c}_{kt}")
                nc.gpsimd.dma_start(xt[:], x_d.ap()[kt, c])
                xc.append(xt)
            xcs.append(xc)

        ones_k = const.tile([128, 1], BF16, name="ones_k")
        nc.vector.memset(ones_k[:], 1.0)
        ones_m = const.tile([1, 128], F32, name="ones_m")
        nc.vector.memset(ones_m[:], 1.0)
        eps_sb = const.tile([1, 1], F32, name="eps_sb")
        nc.vector.memset(eps_sb[:], EPS)

        ub_sb = const.tile([128, 16], F32, name="ub_sb")
        nc.gpsimd.dma_start(ub_sb[:], ub_d.ap()[:, :])
        pb_sb = const.tile([128, 4], F32, name="pb_sb")
        nc.gpsimd.dma_start(pb_sb[:], pb_d.ap()[:, :])
        gb_sb = const.tile([128, 8], F32, name="gb_sb")
        nc.gpsimd.dma_start(gb_sb[:], gb_d.ap()[:, :])

        # ---- gathers first: GPSIMD/SWDGE fills embedding tiles while the
        # rest of the pipeline boots. Gathers for j<k are pre-summed per k
        # (bf16 SBUF adds, cheap and early) so each U psum group later needs
        # only ONE DVE add.
        gpool = ctx.enter_context(tc.tile_pool(name="gpool", bufs=2))

        def gather(k, j, c, tag, name, bufs=None):
            # num_idxs=1024 crashes the exec unit (HW-bisected); issue two
            # 512-idx gathers into the contiguous half-slices of one tile.
            gt = gpool.tile([128, 2, 4, MH], BF16, tag=tag, name=name, bufs=bufs)
            for h in range(2):
                s = (2 * c + h) * (MH // 16)
                nc.gpsimd.dma_gather(
                    out_ap=gt[:, h],
                    in_ap=tab_d[(k, j)].ap(),
                    idxs_ap=idx_sb[j][:, s:s + MH // 16],
                    num_idxs=MH,
                    num_idxs_reg=MH,
                    elem_size=D,
                    transpose=True,
                )
            return gt

        # Gather emission is spread across main-loop k-phases (emission order
        # == descriptor enqueue order): input DMAs get full bandwidth first,
        # and each gather lands at least one phase before its consumer.
        gsum = {}
        graw = {}

        def emit_gathers(c):
            # All 12 gathers enter the SWDGE queue first (their SDMA jobs all
            # in flight), then the pre-sums run on GPSIMD — ordered after the
            # gathers on the same engine, never blocking the DVE stream.
            gsum[(c, 1)] = gather(1, 0, c, "g10", f"g10_{c}")
            gsum[(c, 2)] = gather(2, 0, c, "gsum2", f"gsum2_{c}", bufs=1)
            graw[(c, 21)] = gather(2, 1, c, "g21", f"g21_{c}", bufs=1)
            gsum[(c, 3)] = gather(3, 0, c, "gsum3", f"gsum3_{c}", bufs=1)
            graw[(c, 31)] = gather(3, 1, c, "g31", f"g31_{c}", bufs=1)
            graw[(c, 32)] = gather(3, 2, c, "g32", f"g32_{c}", bufs=1)
            nc.gpsimd.tensor_add(gsum[(c, 2)][:], gsum[(c, 2)][:],
                                 graw[(c, 21)][:])
            nc.gpsimd.tensor_add(gsum[(c, 3)][:], gsum[(c, 3)][:],
                                 graw[(c, 31)][:])
            nc.gpsimd.tensor_add(gsum[(c, 3)][:], gsum[(c, 3)][:],
                                 graw[(c, 32)][:])

        # ---- weights
        wu_sb, wp_sb = [], []
        for kt in range(4):
            w = const.tile([128, 4 * D], BF16, name=f"wu{kt}")
            nc.gpsimd.dma_start(w[:], wu_d.ap()[kt])
            wu_sb.append(w)
            p = const.tile([128, NV], BF16, name=f"wp{kt}")
            nc.gpsimd.dma_start(p[:], wp_d.ap()[kt])
            wp_sb.append(p)

        lnp = ctx.enter_context(tc.tile_pool(name="lnp", bufs=2))
        upool = ctx.enter_context(tc.tile_pool(name="upool", bufs=2))
        opool = ctx.enter_context(tc.tile_pool(name="opool", bufs=2))
        mps = ctx.enter_context(tc.tile_pool(name="mps", bufs=4, space="PSUM"))

        xns = []
        for c in range(NCHUNK):
            # ---------------- LayerNorm for this chunk ----------------
            xc, xbc = xcs[c], []
            for kt in range(4):
                xb = xpool.tile([128, CH], BF16, tag="xb", bufs=4,
                                name=f"xb_{c}_{kt}")
                nc.scalar.copy(xb[:], xc[kt][:])
                xbc.append(xb)

            ps = mps.tile([1, CH], F32, tag="ps", name=f"ps_{c}")
            pq = mps.tile([1, CH], F32, tag="ps", name=f"pq_{c}")
            x2s = []
            for kt in range(4):
                x2 = lnp.tile([128, CH], BF16, tag="x2", bufs=2,
                              name=f"x2_{c}_{kt}")
                nc.vector.tensor_mul(x2[:], xbc[kt][:], xbc[kt][:])
                x2s.append(x2)
            for h in range(2):
                hs = bass.ts(h, MH)
                for kt in range(4):
                    nc.tensor.matmul(ps[:, hs], ones_k[:], xbc[kt][:, hs],
                                     start=kt == 0, stop=kt == 3)
                for kt in range(4):
                    nc.tensor.matmul(pq[:, hs], ones_k[:], x2s[kt][:, hs],
                                     start=kt == 0, stop=kt == 3)

            mu = lnp.tile([1, CH], F32, tag="mu", bufs=1, name=f"mu_{c}")
            nc.vector.tensor_scalar_mul(mu[:], ps[:], 1.0 / D)
            m2 = lnp.tile([1, CH], F32, tag="m2", bufs=1, name=f"m2_{c}")
            nc.vector.tensor_scalar_mul(m2[:], pq[:], 1.0 / D)
            var = lnp.tile([1, CH], F32, tag="var", bufs=1, name=f"var_{c}")
            nc.vector.tensor_mul(var[:], mu[:], mu[:])
            nc.vector.tensor_sub(var[:], m2[:], var[:])
            sd = lnp.tile([1, CH], F32, tag="sd", bufs=1, name=f"sd_{c}")
            nc.scalar.activation(sd[:], var[:], AF.Sqrt, bias=eps_sb[:])
            rstd = lnp.tile([1, CH], F32, tag="rstd", bufs=1, name=f"rstd_{c}")
            nc.vector.reciprocal_approx_fast(rstd[:], sd[:])

            pmu = mps.tile([128, CH], F32, tag="ps", name=f"pmu_{c}")
            prs = mps.tile([128, CH], F32, tag="ps", name=f"prs_{c}")
            for h in range(2):
                hs = bass.ts(h, MH)
                nc.tensor.matmul(pmu[:, hs], ones_m[:], mu[:, hs],
                                 start=True, stop=True)
                nc.tensor.matmul(prs[:, hs], ones_m[:], rstd[:, hs],
                                 start=True, stop=True)

            xnc = []
            for kt in range(4):
                t1 = lnp.tile([128, CH], F32, tag="t1", name=f"t1_{c}_{kt}")
                nc.vector.tensor_sub(t1[:], xc[kt][:], pmu[:])
                t2 = lnp.tile([128, CH], BF16, tag="t2", name=f"t2_{c}_{kt}")
                nc.vector.tensor_mul(t2[:], t1[:], prs[:])
                xn = xpool.tile([128, CH], BF16, tag=f"xn{kt}", name=f"xn_{c}_{kt}")
                nc.vector.tensor_scalar(
                    xn[:], t2[:],
                    gb_sb[:, kt:kt + 1], gb_sb[:, 4 + kt:4 + kt + 1],
                    ALU.mult, ALU.add)
                xnc.append(xn)
            xns.append(xnc)

        # ---------------- U matmul + gather-add + relu + P matmul --------
        # Pre-sums of the per-k gather tiles are emitted just before the
        # k-phase that consumes them: by then their gathers have drained, so
        # the in-order DVE queue never blocks on GPSIMD.
        emit_gathers(0)
        for c in range(NCHUNK):
            xnc = xns[c]
            for k in range(NCH):
                if c + 1 < NCHUNK and k == 3:
                    emit_gathers(c + 1)
                ur = []
                for cc in range(4):
                    mt = k * 4 + cc
                    py = mps.tile([128, CH], F32, tag="ps", name=f"py_{c}_{mt}")
                    for h in range(2):
                        hs = bass.ts(h, MH)
                        for kt in range(4):
                            nc.tensor.matmul(
                                py[:, hs],
                                wu_sb[kt][:, mt * 128:(mt + 1) * 128],
                                xnc[kt][:, hs],
                                start=kt == 0, stop=kt == 3)
                    urt = upool.tile([128, CH], BF16, tag=f"ur{cc}",
                                     name=f"ur_{c}_{mt}")
                    if k > 0:
                        ug = lnp.tile([128, CH], F32, tag="ug", bufs=2,
                                      name=f"ug_{c}_{mt}")
                        nc.vector.tensor_add(
                            ug[:].rearrange("p (a b) -> p a b", a=2),
                            py[:].rearrange("p (a b) -> p a b", a=2),
                            gsum[(c, k)][:, :, cc, :])
                        src = ug
                    else:
                        src = py
                    nc.scalar.activation(urt[:], src[:], AF.Relu,
                                         bias=ub_sb[:, mt:mt + 1])
                    ur.append(urt)

                for mt2 in range(4):
                    po = mps.tile([128, CH], F32, tag="ps", name=f"po_{c}_{k}_{mt2}")
                    for h in range(2):
                        hs = bass.ts(h, MH)
                        for kt2 in range(4):
                            nc.tensor.matmul(
                                po[:, hs],
                                wp_sb[kt2][:, mt2 * 128:(mt2 + 1) * 128],
                                ur[kt2][:, hs],
                                start=kt2 == 0, stop=kt2 == 3)
                    ot = opool.tile([128, CH], F32, tag="ot", name=f"ot_{c}_{k}_{mt2}")
                    nc.scalar.activation(ot[:], po[:], AF.Identity,
                                         bias=pb_sb[:, mt2:mt2 + 1])
                    nc.sync.dma_start(out_d[k].ap()[mt2, c], ot[:])

    nc.compile()
    return nc


def make_in_maps(inputs):
    yl = np.ascontiguousarray(np.asarray(inputs["yl"], np.float32))
    slice_idx = np.asarray(inputs["slice_idx"]).astype(np.int64)
    Uw = [np.asarray(inputs[f"U{k}_w"], np.float32) for k in range(4)]
    Ub = [np.asarray(inputs[f"U{k}_b"], np.float32) for k in range(4)]
    Pw = np.asarray(inputs["P_w"], np.float32)
    Pb = np.asarray(inputs["P_b"], np.float32)
    gamma = np.asarray(inputs["ln_gamma"], np.float32)
    beta = np.asarray(inputs["ln_beta"], np.float32)

    wu = np.ascontiguousarray(
        np.concatenate([Uw[k][:, :D] for k in range(4)], axis=0).T
    ).reshape(4, 128, 4 * D).astype(BF16NP)
    wp = np.ascontiguousarray(Pw.T).reshape(4, 128, NV).astype(BF16NP)
    tabs = {
        (k, j): np.ascontiguousarray(
            Uw[k][:, D + j * NV: D + (j + 1) * NV].T
        ).astype(BF16NP)
        for (k, j) in PAIRS
    }
    ub = np.stack(Ub).reshape(4, 4, 128).transpose(2, 0, 1).reshape(128, 16)
    ub = np.ascontiguousarray(ub, np.float32)
    pb = np.ascontiguousarray(Pb.reshape(4, 128).T, np.float32)
    gb = np.ascontiguousarray(
        np.concatenate([gamma.reshape(4, 128).T, beta.reshape(4, 128).T], axis=1),
        np.float32)

    shared = {"wu": wu, "wp": wp, "ub": ub, "pb": pb, "gb": gb}
    for (k, j), t in tabs.items():
        shared[f"tab{k}{j}"] = t

    in_maps = []
    for core in range(NCORES):
        s0 = core * BLOC
        ylc = yl[s0:s0 + BLOC].reshape(BLOC, D, NTOK)
        x = np.concatenate(list(ylc), axis=1)            # (D, TOK)
        x = np.ascontiguousarray(
            x.reshape(4, 128, NCHUNK, CH).transpose(0, 2, 1, 3))
        idxc = slice_idx[s0:s0 + BLOC].reshape(BLOC, NCH, NTOK)
        idx_tok = np.concatenate([idxc[s] for s in range(BLOC)], axis=1)  # (4, TOK)
        wrapped = idx_tok[:3].reshape(3, TOK // 16, 16).transpose(0, 2, 1)
        wrapped = np.ascontiguousarray(
            np.tile(wrapped, (1, 8, 1)).astype(np.int16))  # (3,128,TOK//16)
        in_maps.append({"x": x, "idx": wrapped, **shared})
    return in_maps


def assemble_outputs(results):
    outs = []
    for k in range(NCH):
        per_core = []
        for core in range(NCORES):
            o = results[core][f"out{k}"]  # (4, NCHUNK, 128, CH)
            o = o.transpose(0, 2, 1, 3).reshape(NV, TOK)  # (512, 2048)
            o = o.reshape(NV, BLOC, NTOK).transpose(1, 0, 2)
            per_core.append(o.reshape(BLOC, NV, T, H, W))
        outs.append(np.ascontiguousarray(np.concatenate(per_core, axis=0),
                                         np.float32))
    return tuple(outs)


_CACHE = {}


def kernel(**inputs):
    if "nc" not in _CACHE:
        _CACHE["nc"] = build_program()
    nc = _CACHE["nc"]
    in_maps = make_in_maps(inputs)
    res = bass_utils.run_bass_kernel_spmd(nc, in_maps, core_ids=list(range(NCORES)))
    return assemble_outputs(res.results)


# revision 40
# speedup vs baseline: 1.3153x; 1.3153x over previous
"""Trainium2 Bass kernel for ChannelPredictor (dense_mlp).

Math (per batch sample, N=t*h*w=1024 tokens, D=NV=512, NC=4 channels):
  x = LayerNorm_d(yl)                       # (N, D), token-major in ref
  for k in 0..3:
    u_k = x @ Uk[:, :D].T + sum_{j<k} Uk[:, D+j*NV + idx_j].T + Uk_b
    o_k = relu(u_k) @ P.T + P_b

Reformulations used here:
  * one-hot @ Uk_w == gather of Uk_w columns -> dma_gather of bf16 embedding
    rows (tables pre-transposed host-side to (vocab, D) row-major), written
    d-major across partitions (transpose=True) so they add directly onto the
    matmul PSUM tiles.
  * the x-parts of all four U_k share rhs x -> one fused (D -> 4D) matmul.
  * LayerNorm stats via ones-vector matmuls (bf16 data, f32 accumulate);
    per-token mu/rstd broadcast across partitions via K=1 fp32 matmuls.

Sharding: data-parallel over batch b: 16 samples / 8 cores = 2 samples
(2048 tokens) per core; every core holds the full weights.

Schedule: all gathers are emitted first (GPSIMD/SWDGE runs them during the
LayerNorm lead-in); LayerNorm and the U/P matmul stages are interleaved per
512-token chunk so the PE pipeline fills early and stays warm.
"""

import sys

for _p in ("/opt/trn_rl_repo",):
    if _p not in sys.path:
        sys.path.insert(0, _p)

import numpy as np
import ml_dtypes
from contextlib import ExitStack

import concourse.bass as bass
import concourse.bacc as bacc
import concourse.mybir as mybir
import concourse.tile as tile
from concourse import bass_utils

F32 = mybir.dt.float32
BF16 = mybir.dt.bfloat16
I16 = mybir.dt.int16
AF = mybir.ActivationFunctionType
ALU = mybir.AluOpType
BF16NP = ml_dtypes.bfloat16

NCORES = 8
B, D, NV, NCH = 16, 512, 512, 4
T, H, W = 4, 16, 16
NTOK = T * H * W          # tokens per sample
BLOC = B // NCORES        # samples per core
TOK = BLOC * NTOK         # tokens per core
CH = 1024                 # token chunk (double-width PSUM tiles)
MH = 512                  # matmul moving-dim half of a chunk
NCHUNK = TOK // CH
EPS = 1e-5
PAIRS = [(1, 0), (2, 0), (2, 1), (3, 0), (3, 1), (3, 2)]  # (k, j) with j < k


def build_program():
    nc = bacc.Bacc("TRN2", target_bir_lowering=False, debug=False)

    x_d = nc.dram_tensor("x", (4, NCHUNK, 128, CH), F32, kind="ExternalInput")
    wu_d = nc.dram_tensor("wu", (4, 128, 4 * D), BF16, kind="ExternalInput")
    wp_d = nc.dram_tensor("wp", (4, 128, NV), BF16, kind="ExternalInput")
    tab_d = {
        (k, j): nc.dram_tensor(f"tab{k}{j}", (NV, D), BF16, kind="ExternalInput")
        for (k, j) in PAIRS
    }
    idx_d = nc.dram_tensor("idx", (3, 128, TOK // 16), I16, kind="ExternalInput")
    ub_d = nc.dram_tensor("ub", (128, 16), F32, kind="ExternalInput")
    pb_d = nc.dram_tensor("pb", (128, 4), F32, kind="ExternalInput")
    gb_d = nc.dram_tensor("gb", (128, 8), F32, kind="ExternalInput")
    out_d = [
        nc.dram_tensor(f"out{k}", (4, NCHUNK, 128, CH), F32, kind="ExternalOutput")
        for k in range(NCH)
    ]

    with tile.TileContext(nc) as tc, ExitStack() as ctx:
        const = ctx.enter_context(tc.tile_pool(name="const", bufs=1))

        idx_sb = []
        for j in range(3):
            it = const.tile([128, TOK // 16], I16, name=f"idx{j}")
            nc.gpsimd.dma_start(it[:], idx_d.ap()[j])
            idx_sb.append(it)

        # x loads first: the LayerNorm lead-in is the critical path.
        xpool = ctx.enter_context(tc.tile_pool(name="xpool", bufs=2))
        xcs = []
        for c in range(NCHUNK):
            xc = []
            for kt in range(4):
                xt = xpool.tile([128, CH], F32, tag=f"x{kt}", name=f"x_{c}_{kt}")
                nc.gpsimd.dma_start(xt[:], x_d.ap()[kt, c])
                xc.append(xt)
            xcs.append(xc)

        ones_k = const.tile([128, 1], BF16, name="ones_k")
        nc.vector.memset(ones_k[:], 1.0)
        ones_m = const.tile([1, 128], F32, name="ones_m")
        nc.vector.memset(ones_m[:], 1.0)
        eps_sb = const.tile([1, 1], F32, name="eps_sb")
        nc.vector.memset(eps_sb[:], EPS)

        ub_sb = const.tile([128, 16], F32, name="ub_sb")
        nc.gpsimd.dma_start(ub_sb[:], ub_d.ap()[:, :])
        pb_sb = const.tile([128, 4], F32, name="pb_sb")
        nc.gpsimd.dma_start(pb_sb[:], pb_d.ap()[:, :])
        gb_sb = const.tile([128, 8], F32, name="gb_sb")
        nc.gpsimd.dma_start(gb_sb[:], gb_d.ap()[:, :])

        # ---- gathers first: GPSIMD/SWDGE fills embedding tiles while the
        # rest of the pipeline boots. Gathers for j<k are pre-summed per k
        # (bf16 SBUF adds, cheap and early) so each U psum group later needs
        # only ONE DVE add.
        gpool = ctx.enter_context(tc.tile_pool(name="gpool", bufs=2))

        def gather(k, j, c, tag, name, bufs=None):
            # num_idxs=1024 crashes the exec unit (HW-bisected); issue two
            # 512-idx gathers into the contiguous half-slices of one tile.
            gt = gpool.tile([128, 2, 4, MH], BF16, tag=tag, name=name, bufs=bufs)
            for h in range(2):
                s = (2 * c + h) * (MH // 16)
                nc.gpsimd.dma_gather(
                    out_ap=gt[:, h],
                    in_ap=tab_d[(k, j)].ap(),
                    idxs_ap=idx_sb[j][:, s:s + MH // 16],
                    num_idxs=MH,
                    num_idxs_reg=MH,
                    elem_size=D,
                    transpose=True,
                )
            return gt

        # Gather emission is spread across main-loop k-phases (emission order
        # == descriptor enqueue order): input DMAs get full bandwidth first,
        # and each gather lands at least one phase before its consumer.
        gparts = {}

        def emit_gathers(c):
            # Raw per-(k,j) tiles; summed per U group on DVE (the adds wait
            # only on their own inputs, so they cannot head-of-line-block).
            gparts[(c, 1)] = [gather(1, 0, c, "g10", f"g10_{c}")]
            gparts[(c, 2)] = [gather(2, 0, c, "g20", f"g20_{c}", bufs=1),
                              gather(2, 1, c, "g21", f"g21_{c}", bufs=1)]
            gparts[(c, 3)] = [gather(3, 0, c, "g30", f"g30_{c}", bufs=1),
                              gather(3, 1, c, "g31", f"g31_{c}", bufs=1),
                              gather(3, 2, c, "g32", f"g32_{c}", bufs=1)]

        # ---- weights
        wu_sb, wp_sb = [], []
        for kt in range(4):
            w = const.tile([128, 4 * D], BF16, name=f"wu{kt}")
            nc.gpsimd.dma_start(w[:], wu_d.ap()[kt])
            wu_sb.append(w)
            p = const.tile([128, NV], BF16, name=f"wp{kt}")
            nc.gpsimd.dma_start(p[:], wp_d.ap()[kt])
            wp_sb.append(p)

        lnp = ctx.enter_context(tc.tile_pool(name="lnp", bufs=2))
        upool = ctx.enter_context(tc.tile_pool(name="upool", bufs=2))
        opool = ctx.enter_context(tc.tile_pool(name="opool", bufs=2))
        mps = ctx.enter_context(tc.tile_pool(name="mps", bufs=4, space="PSUM"))

        xns = []
        for c in range(NCHUNK):
            # ---------------- LayerNorm for this chunk ----------------
            xc, xbc = xcs[c], []
            for kt in range(4):
                xb = xpool.tile([128, CH], BF16, tag="xb", bufs=4,
                                name=f"xb_{c}_{kt}")
                nc.scalar.copy(xb[:], xc[kt][:])
                xbc.append(xb)

            ps = mps.tile([1, CH], F32, tag="ps", name=f"ps_{c}")
            pq = mps.tile([1, CH], F32, tag="ps", name=f"pq_{c}")
            x2s = []
            for kt in range(4):
                x2 = lnp.tile([128, CH], BF16, tag="x2", bufs=2,
                              name=f"x2_{c}_{kt}")
                nc.vector.tensor_mul(x2[:], xbc[kt][:], xbc[kt][:])
                x2s.append(x2)
            for h in range(2):
                hs = bass.ts(h, MH)
                for kt in range(4):
                    nc.tensor.matmul(ps[:, hs], ones_k[:], xbc[kt][:, hs],
                                     start=kt == 0, stop=kt == 3)
                for kt in range(4):
                    nc.tensor.matmul(pq[:, hs], ones_k[:], x2s[kt][:, hs],
                                     start=kt == 0, stop=kt == 3)

            mu = lnp.tile([1, CH], F32, tag="mu", bufs=1, name=f"mu_{c}")
            nc.vector.tensor_scalar_mul(mu[:], ps[:], 1.0 / D)
            m2 = lnp.tile([1, CH], F32, tag="m2", bufs=1, name=f"m2_{c}")
            nc.vector.tensor_scalar_mul(m2[:], pq[:], 1.0 / D)
            var = lnp.tile([1, CH], F32, tag="var", bufs=1, name=f"var_{c}")
            nc.vector.tensor_mul(var[:], mu[:], mu[:])
            nc.vector.tensor_sub(var[:], m2[:], var[:])
            sd = lnp.tile([1, CH], F32, tag="sd", bufs=1, name=f"sd_{c}")
            nc.scalar.activation(sd[:], var[:], AF.Sqrt, bias=eps_sb[:])
            rstd = lnp.tile([1, CH], F32, tag="rstd", bufs=1, name=f"rstd_{c}")
            nc.vector.reciprocal_approx_fast(rstd[:], sd[:])

            pmu = mps.tile([128, CH], F32, tag="ps", name=f"pmu_{c}")
            prs = mps.tile([128, CH], F32, tag="ps", name=f"prs_{c}")
            for h in range(2):
                hs = bass.ts(h, MH)
                nc.tensor.matmul(pmu[:, hs], ones_m[:], mu[:, hs],
                                 start=True, stop=True)
                nc.tensor.matmul(prs[:, hs], ones_m[:], rstd[:, hs],
                                 start=True, stop=True)

            xnc = []
            for kt in range(4):
                t1 = lnp.tile([128, CH], F32, tag="t1", name=f"t1_{c}_{kt}")
                nc.vector.tensor_sub(t1[:], xc[kt][:], pmu[:])
                t2 = lnp.tile([128, CH], BF16, tag="t2", name=f"t2_{c}_{kt}")
                nc.vector.tensor_mul(t2[:], t1[:], prs[:])
                xn = xpool.tile([128, CH], BF16, tag=f"xn{kt}", name=f"xn_{c}_{kt}")
                nc.vector.tensor_scalar(
                    xn[:], t2[:],
                    gb_sb[:, kt:kt + 1], gb_sb[:, 4 + kt:4 + kt + 1],
                    ALU.mult, ALU.add)
                xnc.append(xn)
            xns.append(xnc)

        # ---------------- U matmul + gather-add + relu + P matmul --------
        # Pre-sums of the per-k gather tiles are emitted just before the
        # k-phase that consumes them: by then their gathers have drained, so
        # the in-order DVE queue never blocks on GPSIMD.
        emit_gathers(0)
        for c in range(NCHUNK):
            xnc = xns[c]
            for k in range(NCH):
                if c + 1 < NCHUNK and k == 1:
                    emit_gathers(c + 1)
                ur = []
                for cc in range(4):
                    mt = k * 4 + cc
                    py = mps.tile([128, CH], F32, tag="ps", name=f"py_{c}_{mt}")
                    for h in range(2):
                        hs = bass.ts(h, MH)
                        for kt in range(4):
                            nc.tensor.matmul(
                                py[:, hs],
                                wu_sb[kt][:, mt * 128:(mt + 1) * 128],
                                xnc[kt][:, hs],
                                start=kt == 0, stop=kt == 3)
                    urt = upool.tile([128, CH], BF16, tag=f"ur{cc}",
                                     name=f"ur_{c}_{mt}")
                    if k > 0:
                        acc = py[:].rearrange("p (a b) -> p a b", a=2)
                        for gj, gt in enumerate(gparts[(c, k)]):
                            ug = lnp.tile([128, CH], F32, tag="ug", bufs=3,
                                          name=f"ug_{c}_{mt}_{gj}")
                            ugv = ug[:].rearrange("p (a b) -> p a b", a=2)
                            nc.vector.tensor_add(ugv, acc, gt[:, :, cc, :])
                            acc = ugv
                        src = ug
                    else:
                        src = py
                    nc.scalar.activation(urt[:], src[:], AF.Relu,
                                         bias=ub_sb[:, mt:mt + 1])
                    ur.append(urt)

                for mt2 in range(4):
                    po = mps.tile([128, CH], F32, tag="ps", name=f"po_{c}_{k}_{mt2}")
                    for h in range(2):
                        hs = bass.ts(h, MH)
                        for kt2 in range(4):
                            nc.tensor.matmul(
                                po[:, hs],
                                wp_sb[kt2][:, mt2 * 128:(mt2 + 1) * 128],
                                ur[kt2][:, hs],
                                start=kt2 == 0, stop=kt2 == 3)
                    ot = opool.tile([128, CH], F32, tag="ot", name=f"ot_{c}_{k}_{mt2}")
                    nc.scalar.activation(ot[:], po[:], AF.Identity,
                                         bias=pb_sb[:, mt2:mt2 + 1])
                    nc.sync.dma_start(out_d[k].ap()[mt2, c], ot[:])

    nc.compile()
    return nc


def make_in_maps(inputs):
    yl = np.ascontiguousarray(np.asarray(inputs["yl"], np.float32))
    slice_idx = np.asarray(inputs["slice_idx"]).astype(np.int64)
    Uw = [np.asarray(inputs[f"U{k}_w"], np.float32) for k in range(4)]
    Ub = [np.asarray(inputs[f"U{k}_b"], np.float32) for k in range(4)]
    Pw = np.asarray(inputs["P_w"], np.float32)
    Pb = np.asarray(inputs["P_b"], np.float32)
    gamma = np.asarray(inputs["ln_gamma"], np.float32)
    beta = np.asarray(inputs["ln_beta"], np.float32)

    wu = np.ascontiguousarray(
        np.concatenate([Uw[k][:, :D] for k in range(4)], axis=0).T
    ).reshape(4, 128, 4 * D).astype(BF16NP)
    wp = np.ascontiguousarray(Pw.T).reshape(4, 128, NV).astype(BF16NP)
    tabs = {
        (k, j): np.ascontiguousarray(
            Uw[k][:, D + j * NV: D + (j + 1) * NV].T
        ).astype(BF16NP)
        for (k, j) in PAIRS
    }
    ub = np.stack(Ub).reshape(4, 4, 128).transpose(2, 0, 1).reshape(128, 16)
    ub = np.ascontiguousarray(ub, np.float32)
    pb = np.ascontiguousarray(Pb.reshape(4, 128).T, np.float32)
    gb = np.ascontiguousarray(
        np.concatenate([gamma.reshape(4, 128).T, beta.reshape(4, 128).T], axis=1),
        np.float32)

    shared = {"wu": wu, "wp": wp, "ub": ub, "pb": pb, "gb": gb}
    for (k, j), t in tabs.items():
        shared[f"tab{k}{j}"] = t

    in_maps = []
    for core in range(NCORES):
        s0 = core * BLOC
        ylc = yl[s0:s0 + BLOC].reshape(BLOC, D, NTOK)
        x = np.concatenate(list(ylc), axis=1)            # (D, TOK)
        x = np.ascontiguousarray(
            x.reshape(4, 128, NCHUNK, CH).transpose(0, 2, 1, 3))
        idxc = slice_idx[s0:s0 + BLOC].reshape(BLOC, NCH, NTOK)
        idx_tok = np.concatenate([idxc[s] for s in range(BLOC)], axis=1)  # (4, TOK)
        wrapped = idx_tok[:3].reshape(3, TOK // 16, 16).transpose(0, 2, 1)
        wrapped = np.ascontiguousarray(
            np.tile(wrapped, (1, 8, 1)).astype(np.int16))  # (3,128,TOK//16)
        in_maps.append({"x": x, "idx": wrapped, **shared})
    return in_maps


def assemble_outputs(results):
    outs = []
    for k in range(NCH):
        per_core = []
        for core in range(NCORES):
            o = results[core][f"out{k}"]  # (4, NCHUNK, 128, CH)
            o = o.transpose(0, 2, 1, 3).reshape(NV, TOK)  # (512, 2048)
            o = o.reshape(NV, BLOC, NTOK).transpose(1, 0, 2)
            per_core.append(o.reshape(BLOC, NV, T, H, W))
        outs.append(np.ascontiguousarray(np.concatenate(per_core, axis=0),
                                         np.float32))
    return tuple(outs)


_CACHE = {}


def kernel(**inputs):
    if "nc" not in _CACHE:
        _CACHE["nc"] = build_program()
    nc = _CACHE["nc"]
    in_maps = make_in_maps(inputs)
    res = bass_utils.run_bass_kernel_spmd(nc, in_maps, core_ids=list(range(NCORES)))
    return assemble_outputs(res.results)
